# revision 5
# baseline (speedup 1.0000x reference)
"""GCNBlock (GCNConv + BatchNorm1d eval + ReLU) on 8 Trainium2 NeuronCores.

Strategy
--------
out = ReLU(BN(D^-1/2 (A+I) D^-1/2 (X W) + b))   with in-degree-based D.

Algebraic folding (host, cheap O(N*IN) / O(OC) work):
  sc  = gamma * rsqrt(var + eps)                    [OC]
  W2  = W * sc                                      [IN, OC]  f32
  c2  = beta + (b - mean) * sc                      [OC]
  dis = rsqrt(deg), deg = indeg + 1                 [N]
  gx  = x * dis[:, None]  (fp16)                    [N, IN]
  self-loops become plain edges (n, n): their weight dis[n]^2 = 1/deg[n]
  matches the reference self-loop term exactly.

Then  out^T = ReLU(W2^T @ T^T + c2),  T^T[:, n] = sum_{e: dst=n} dis[dst] * gx[src].

Device (per core, dst-node shard of 12500 nodes):
  - edges sorted by (128-node subwindow, src-range k) on host; padded to
    128-edge chunks; all chunk counts equalized across cores (max) so one
    SPMD program serves all 8 cores.
  - dma_gather pulls 128 gx rows (256B fp16) per chunk from HBM.
  - DVE builds a dis-weighted one-hot S = (iota == dstloc) * disE per chunk.
  - PE accumulates T^T[128dims, 128nodes] += Gx_chunk^T @ S in PSUM.
  - per subwindow: T^T -> SBUF, second matmul W2^T @ T^T -> [64, 128],
    ACT epilogue ReLU(x + c2), staged and DMA'd out transposed.
  - host transposes the assembled [64, 100000] to [100000, 64].
"""

import os
import sys

sys.path.insert(0, "/opt/trn_rl_repo")

import numpy as np

N_NODES = 100000
N_EDGES = 1600000
IN_DIM = 128
OUT_DIM = 64
BN_EPS = 1e-5

NCORES = 8
SHARD = N_NODES // NCORES            # 12500 dst nodes per core
P = 128
NSUB = (SHARD + P - 1) // P          # 98 subwindows (last has 84 nodes)
KS = 25600                           # src-range size (< int16 max 32767)
NK = (N_NODES + KS - 1) // KS        # 4 src ranges
GROUP_SUBS = 8
NGROUP = (NSUB + GROUP_SUBS - 1) // GROUP_SUBS   # 13

TRACE = False            # set True from test harness to profile
LAST_RESULT = {}         # exec_time_ns etc. stashed here when TRACE


def _host_schedule(src, dst):
    """Sort/augment edges, compute the uniform chunk schedule.

    Returns per-core arrays (idx16 wrapped, dstloc cols, disE cols) plus the
    shared CH[sub, k] chunk-count table and layout offsets.
    """
    n_aug = N_EDGES + N_NODES
    src_a = np.empty(n_aug, dtype=np.int64)
    dst_a = np.empty(n_aug, dtype=np.int64)
    src_a[:N_EDGES] = src
    dst_a[:N_EDGES] = dst
    arange_n = np.arange(N_NODES, dtype=np.int64)
    src_a[N_EDGES:] = arange_n
    dst_a[N_EDGES:] = arange_n

    core = dst_a // SHARD
    rel = dst_a - core * SHARD
    sub = rel >> 7
    k = src_a // KS

    order = np.lexsort((k, sub, core))
    src_s = src_a[order]
    dst_s = dst_a[order]
    core_s = core[order]
    sub_s = sub[order]
    k_s = k[order]
    dstlow_s = (rel[order] & 127).astype(np.float32)

    grp = (core_s * NSUB + sub_s) * NK + k_s        # sorted ascending
    counts_flat = np.bincount(grp, minlength=NCORES * NSUB * NK)
    counts = counts_flat.reshape(NCORES, NSUB, NK)

    CH = -(-counts.max(axis=0) // P)                # [NSUB, NK] chunk counts
    # column layout: for g: for k: for s in group g
    colstart = np.zeros((NSUB, NK), dtype=np.int64)
    callinfo = {}                                   # (g,k) -> (col_off, cols)
    off = 0
    for g in range(NGROUP):
        subs_g = range(g * GROUP_SUBS, min(NSUB, (g + 1) * GROUP_SUBS))
        for kk in range(NK):
            cols = 0
            base = off
            for s in subs_g:
                colstart[s, kk] = off
                off += CH[s, kk]
                cols += CH[s, kk]
            callinfo[(g, kk)] = (base, cols)
    chtot = off
    idxtot = chtot * P

    # position of each edge in the padded chunk stream of its core
    seg_counts = counts_flat[grp[np.r_[0, np.flatnonzero(np.diff(grp)) + 1]]] \
        if len(grp) else np.array([], dtype=np.int64)
    seg_start = np.r_[0, np.cumsum(seg_counts)[:-1]]
    cumcount = np.arange(n_aug, dtype=np.int64) - np.repeat(seg_start, seg_counts)
    pos = colstart[sub_s, k_s] * P + cumcount       # within-core position

    idxloc_s = (src_s - k_s * KS).astype(np.int16)

    return (core_s, pos, idxloc_s, dstlow_s, dst_s,
            CH, colstart, callinfo, chtot, idxtot)


def _build_core_arrays(core_s, pos, idxloc_s, dstlow_s, dis_dst_s,
                       chtot, idxtot):
    """Scatter sorted edge attrs into the uniform padded layout per core."""
    out = []
    for c in range(NCORES):
        m = core_s == c
        p = pos[m]
        idx_flat = np.zeros(idxtot, dtype=np.int16)
        dstloc_flat = np.full(idxtot, -1.0, dtype=np.float32)
        dise_flat = np.zeros(idxtot, dtype=np.float16)
        idx_flat[p] = idxloc_s[m]
        dstloc_flat[p] = dstlow_s[m]
        dise_flat[p] = dis_dst_s[m]
        idx_w = idx_flat.reshape(idxtot // 16, 16).T          # [16, idxtot/16]
        idx_rep = np.tile(idx_w, (8, 1)).copy()               # [128, idxtot/16]
        dst_cols = dstloc_flat.reshape(chtot, P).T.copy()     # [128, chtot]
        dise_cols = dise_flat.reshape(chtot, P).T.copy()      # [128, chtot]
        out.append((idx_rep, dst_cols, dise_cols))
    return out


def _build_program(CH, colstart, callinfo, chtot, idxtot):
    import concourse.bacc as bacc
    import concourse.mybir as mybir
    import concourse.tile as tile
    from concourse.library_config import mlp

    nc = bacc.Bacc("TRN2", debug=False)
    f16, f32, i16 = mybir.dt.float16, mybir.dt.float32, mybir.dt.int16
    t_gx = nc.dram_tensor("gx", [N_NODES, IN_DIM], f16, kind="ExternalInput")
    t_idx = nc.dram_tensor("idx", [P, idxtot // 16], i16, kind="ExternalInput")
    t_dst = nc.dram_tensor("dstloc", [P, chtot], f32, kind="ExternalInput")
    t_dis = nc.dram_tensor("disE", [P, chtot], f16, kind="ExternalInput")
    t_iota = nc.dram_tensor("iota", [P, P], f16, kind="ExternalInput")
    t_w2 = nc.dram_tensor("w2", [IN_DIM, OUT_DIM], f32, kind="ExternalInput")
    t_c2 = nc.dram_tensor("c2", [OUT_DIM, 1], f32, kind="ExternalInput")
    OUTCOLS = NGROUP * GROUP_SUBS * P               # 13312
    t_out = nc.dram_tensor("out", [OUT_DIM, OUTCOLS], f32, kind="ExternalOutput")

    with tile.TileContext(nc) as tc:
        with (
            tc.tile_pool(name="pconst", bufs=1) as pconst,
            tc.tile_pool(name="pgb", bufs=2) as pgb,
            tc.tile_pool(name="psel", bufs=6) as psel,
            tc.tile_pool(name="ppt", bufs=3) as ppt,
            tc.tile_pool(name="pobuf", bufs=2) as pobuf,
            tc.tile_pool(name="pacc", bufs=2, space="PSUM") as pacc,
            tc.tile_pool(name="pp2", bufs=2, space="PSUM") as pp2,
        ):
            nc.gpsimd.load_library(mlp)
            idx_t = pconst.tile([P, idxtot // 16], i16)
            nc.sync.dma_start(idx_t[:], t_idx[:])
            dst_t = pconst.tile([P, chtot], f32)
            nc.sync.dma_start(dst_t[:], t_dst[:])
            dis_t = pconst.tile([P, chtot], f16)
            nc.sync.dma_start(dis_t[:], t_dis[:])
            iota_t = pconst.tile([P, P], f16)
            nc.sync.dma_start(iota_t[:], t_iota[:])
            w2_t = pconst.tile([IN_DIM, OUT_DIM], f32)
            nc.sync.dma_start(w2_t[:], t_w2[:])
            c2_t = pconst.tile([OUT_DIM, 1], f32)
            nc.sync.dma_start(c2_t[:], t_c2[:])

            gbmax = [0] * NK
            for (g, kk), (_, cols) in callinfo.items():
                gbmax[kk] = max(gbmax[kk], cols)

            for g in range(NGROUP):
                subs_g = list(range(g * GROUP_SUBS, min(NSUB, (g + 1) * GROUP_SUBS)))
                gb = {}
                for kk in range(NK):
                    col_off, cols = callinfo[(g, kk)]
                    if cols == 0:
                        continue
                    gt = pgb.tile([P, gbmax[kk], IN_DIM], f16, tag=f"gb{kk}")
                    k0 = kk * KS
                    k1 = min(N_NODES, k0 + KS)
                    # single_packet=True crashes above ~1024 idxs; multi-packet
                    # verified up to 8192 idxs per call.
                    assert cols * P <= 8192, cols
                    nc.gpsimd.dma_gather(
                        gt[:, :cols, :],
                        t_gx[k0:k1, :],
                        idx_t[:, col_off * 8 : (col_off + cols) * 8],
                        cols * P,
                        cols * P,
                        IN_DIM,
                        single_packet=False,
                    )
                    gb[kk] = (gt, col_off)

                obuf = pobuf.tile([OUT_DIM, GROUP_SUBS * P], f32, tag="obuf")
                for si, s in enumerate(subs_g):
                    total = int(CH[s].sum())
                    if total == 0:
                        continue
                    psum = pacc.tile([P, P], f32, tag="acc")
                    done = 0
                    for kk in range(NK):
                        if CH[s, kk] == 0:
                            continue
                        gt, col_off = gb[kk]
                        local = colstart[s, kk] - col_off
                        for i in range(int(CH[s, kk])):
                            q = int(colstart[s, kk]) + i
                            s_t = psel.tile([P, P], f16, tag="s")
                            nc.vector.scalar_tensor_tensor(
                                out=s_t[:],
                                in0=iota_t[:],
                                scalar=dst_t[:, q : q + 1],
                                in1=dis_t[:, q : q + 1].to_broadcast([P, P]),
                                op0=mybir.AluOpType.is_equal,
                                op1=mybir.AluOpType.mult,
                            )
                            done += 1
                            nc.tensor.matmul(
                                out=psum[:],
                                lhsT=gt[:, local + i, :],
                                rhs=s_t[:],
                                start=(done == 1),
                                stop=(done == total),
                            )
                    pt = ppt.tile([P, P], f32, tag="pt")
                    nc.scalar.copy(out=pt[:], in_=psum[:])
                    psum2 = pp2.tile([OUT_DIM, P], f32, tag="p2")
                    nc.tensor.matmul(
                        out=psum2[:], lhsT=w2_t[:], rhs=pt[:], start=True, stop=True
                    )
                    nc.scalar.activation(
                        out=obuf[:, si * P : (si + 1) * P],
                        in_=psum2[:],
                        func=mybir.ActivationFunctionType.Relu,
                        bias=c2_t[:],
                        scale=1.0,
                    )
                nsg = len(subs_g)
                nc.sync.dma_start(
                    t_out[:, g * GROUP_SUBS * P : g * GROUP_SUBS * P + nsg * P],
                    obuf[:, : nsg * P],
                )

    nc.compile()
    return nc


def kernel(x, edge_index, W, b, gamma, beta, run_mean, run_var):
    from concourse.bass_utils import run_bass_kernel_spmd

    x = np.asarray(x, dtype=np.float32)
    edge_index = np.asarray(edge_index)
    src = np.asarray(edge_index[0], dtype=np.int64)
    dst = np.asarray(edge_index[1], dtype=np.int64)
    W = np.asarray(W, dtype=np.float32)
    b = np.asarray(b, dtype=np.float32)
    gamma = np.asarray(gamma, dtype=np.float32)
    beta = np.asarray(beta, dtype=np.float32)
    run_mean = np.asarray(run_mean, dtype=np.float32)
    run_var = np.asarray(run_var, dtype=np.float32)

    deg = (np.bincount(dst, minlength=N_NODES) + 1.0).astype(np.float32)
    dis = (1.0 / np.sqrt(deg)).astype(np.float32)
    gx = (x * dis[:, None]).astype(np.float16)
    sc = gamma / np.sqrt(run_var + BN_EPS)
    W2 = (W * sc[None, :]).astype(np.float32)
    c2 = (beta + (b - run_mean) * sc).astype(np.float32)

    (core_s, pos, idxloc_s, dstlow_s, dst_s,
     CH, colstart, callinfo, chtot, idxtot) = _host_schedule(src, dst)

    dis_dst_s = dis[dst_s].astype(np.float16)

    per_core = _build_core_arrays(core_s, pos, idxloc_s, dstlow_s, dis_dst_s,
                                  chtot, idxtot)

    nc = _build_program(CH, colstart, callinfo, chtot, idxtot)

    iota_np = np.broadcast_to(
        np.arange(P, dtype=np.float16), (P, P)
    ).copy()
    in_maps = []
    for c in range(NCORES):
        idx_rep, dst_cols, dise_cols = per_core[c]
        in_maps.append({
            "gx": gx,
            "idx": idx_rep,
            "dstloc": dst_cols,
            "disE": dise_cols,
            "iota": iota_np,
            "w2": W2,
            "c2": c2[:, None].copy(),
        })

    core_ids = list(range(NCORES))
    res = run_bass_kernel_spmd(nc, in_maps, core_ids, trace=TRACE)
    LAST_RESULT["exec_time_ns"] = res.exec_time_ns
    LAST_RESULT["profile_json"] = res.profile_json

    outT = np.empty((OUT_DIM, N_NODES), dtype=np.float32)
    for c in range(NCORES):
        outT[:, c * SHARD : (c + 1) * SHARD] = res.results[c]["out"][:, :SHARD]
    return np.ascontiguousarray(outT.T)
